# revision 6
# baseline (speedup 1.0000x reference)
"""BoxFilter kernel for Trainium2 (8 NeuronCores).

Computes out[b,0,i,j] = sum_{c} sum_{|di|<=15} sum_{|dj|<=15} x[b,c,i+di,j+dj]
(edge-clamped 31x31 box filter over the channel-summed image), matching the
reference cumsum + shifted-diff formulation exactly (separable box sums).

Sharding: data-parallel over (batch, H-half) -> 8 shards, no cross-core
communication. Each core receives a host-padded [3, 1056, 2048] slab
(16 halo rows on each side, zero-filled past the global image edges).

Per-core pipeline (all f32):
  1. channel-sum on DVE (2 adds per 128-row tile)
  2. vertical 31-tap box sum via two banded 0/1-matrix fp32 matmuls per
     PSUM bank (bands are compile-time constant inputs)
  3. ACT copies PSUM -> zero-padded SBUF tile
  4. horizontal 31-tap box sum in a single tensor_tensor_scan:
     state_j = state_{j-1} + xp[j] - xp[j-31]
  5. DMA result rows to DRAM
"""

import numpy as np

R = 15
TAP = 2 * R + 1          # 31
B, C, H, W = 4, 3, 2048, 2048
HALF = H // 2            # 1024 output rows per core
S_ROWS = HALF + 32       # 1056 input rows per core (16-row halo each side)
N_CORES = 8
PAD_L = TAP              # left zero pad for the scan (31)
PAD_R = R                # right zero pad (15)
XP_W = PAD_L + W + PAD_R # 2094
SCAN_N = W + R           # 2063 scan steps; out col j = scan[j + R]
P = 128                  # SBUF partitions
N_OUT_TILES = HALF // P  # 8
TAIL_ROWS = S_ROWS - N_OUT_TILES * P  # 32 valid rows in the 9th s-tile
MM_N = 512               # fp32 moving-operand max / one PSUM bank

_CACHE = {}


def _band_matrices():
    # out row i of a 128-row tile needs halo'd input rows r = i+1 .. i+31
    # (r is the row index within the [s_lo; s_hi] 256-row window).
    k = np.arange(P)[:, None]
    i = np.arange(P)[None, :]
    band_a = ((k >= i + 1) & (k <= i + TAP)).astype(np.float32)          # rows in s_lo
    band_b = ((k + P >= i + 1) & (k + P <= i + TAP)).astype(np.float32)  # rows in s_hi
    return band_a, band_b


def _build_kernel(tc, nc, out, xs, band_a_d, band_b_d, mybir, bass):
    from contextlib import ExitStack

    f32 = mybir.dt.float32
    f32r = mybir.dt.float32r
    add = mybir.AluOpType.add
    sub = mybir.AluOpType.subtract

    with ExitStack() as ctx:
        const_pool = ctx.enter_context(tc.tile_pool(name="const", bufs=1))
        xc_pool = ctx.enter_context(tc.tile_pool(name="xc", bufs=3))
        s_pool = ctx.enter_context(tc.tile_pool(name="s", bufs=4))
        xp_pool = ctx.enter_context(tc.tile_pool(name="xp", bufs=3))
        box_pool = ctx.enter_context(tc.tile_pool(name="box", bufs=3))
        psum_pool = ctx.enter_context(
            tc.tile_pool(name="psum", bufs=8, space=bass.MemorySpace.PSUM)
        )

        band_a = const_pool.tile([P, P], f32r)
        band_b = const_pool.tile([P, P], f32r)
        nc.sync.dma_start(band_a[:], band_a_d)
        nc.sync.dma_start(band_b[:], band_b_d)

        def make_s(u):
            rows = P if u < N_OUT_TILES else TAIL_ROWS
            # one batched DMA for all 3 channels: [rows, 3, W], partition-major
            xc = xc_pool.tile([P, C, W], f32)
            nc.sync.dma_start(
                xc[:rows],
                xs[:, P * u : P * u + rows, :].rearrange("c p n -> p c n"),
            )
            s = s_pool.tile([P, W], f32r)
            if rows < P:
                # rows >= TAIL_ROWS are multiplied by zero band weights but
                # must be finite, and rows 31.. are simply past the image.
                nc.gpsimd.memset(s[:].bitcast(f32), 0.0)
            nc.gpsimd.tensor_add(s[:rows, :], xc[:rows, 0, :], xc[:rows, 1, :])
            nc.gpsimd.tensor_add(s[:rows, :], s[:rows, :], xc[:rows, 2, :])
            return s

        s_tiles = {0: make_s(0)}
        for t in range(N_OUT_TILES):
            s_tiles[t + 1] = make_s(t + 1)
            s_lo, s_hi = s_tiles.pop(t), s_tiles[t + 1]

            xp = xp_pool.tile([P, XP_W], f32)
            nc.vector.memset(xp[:, 0:PAD_L], 0.0)
            nc.vector.memset(xp[:, PAD_L + W : XP_W], 0.0)

            # all band_a matmuls, then all band_b: minimizes PE weight reloads
            psums = []
            for nb in range(W // MM_N):
                ps = psum_pool.tile([P, MM_N], f32)
                lo_c = s_lo[:, MM_N * nb : MM_N * (nb + 1)]
                nc.tensor.matmul(
                    ps[:], band_a[:], lo_c, start=True, stop=False
                )
                psums.append(ps)
            for nb in range(W // MM_N):
                hi_c = s_hi[:, MM_N * nb : MM_N * (nb + 1)]
                nc.tensor.matmul(
                    psums[nb][:], band_b[:], hi_c,
                    start=False, stop=True,
                )
                nc.scalar.copy(
                    xp[:, PAD_L + MM_N * nb : PAD_L + MM_N * (nb + 1)],
                    psums[nb][:],
                )

            box = box_pool.tile([P, SCAN_N + 1], f32)
            nc.vector.tensor_tensor_scan(
                box[:, 0:SCAN_N],
                xp[:, PAD_L : PAD_L + SCAN_N],
                xp[:, 0:SCAN_N],
                0.0,
                add,
                sub,
            )
            nc.sync.dma_start(out[P * t : P * (t + 1), :], box[:, R : R + W])


def _get_nc():
    if "nc" in _CACHE:
        return _CACHE["nc"]
    import concourse.bass as bass
    import concourse.tile as tile
    from concourse import bacc, mybir

    nc = bacc.Bacc(
        "TRN2", target_bir_lowering=False, debug=False, num_devices=N_CORES
    )
    xs = nc.dram_tensor("xs", [C, S_ROWS, W], mybir.dt.float32, kind="ExternalInput")
    ba = nc.dram_tensor("band_a", [P, P], mybir.dt.float32r, kind="ExternalInput")
    bb = nc.dram_tensor("band_b", [P, P], mybir.dt.float32r, kind="ExternalInput")
    out = nc.dram_tensor("out", [HALF, W], mybir.dt.float32, kind="ExternalOutput")

    with tile.TileContext(nc) as tc:
        _build_kernel(tc, nc, out.ap(), xs.ap(), ba.ap(), bb.ap(), mybir, bass)
    nc.compile()
    _CACHE["nc"] = nc
    return nc


def _in_maps(x):
    band_a, band_b = _band_matrices()
    maps = []
    for k in range(N_CORES):
        b, half = divmod(k, 2)
        h0 = half * HALF
        lo = h0 - 16  # global row of xs row 0
        g0, g1 = max(lo, 0), min(h0 + HALF + 16, H)
        xs = np.zeros((C, S_ROWS, W), np.float32)
        xs[:, g0 - lo : g1 - lo, :] = x[b, :, g0:g1, :]
        maps.append({"xs": xs, "band_a": band_a, "band_b": band_b})
    return maps


def _run(x, trace=False, tmpdir=None):
    from concourse.bass_utils import run_bass_kernel_spmd

    nc = _get_nc()
    res = run_bass_kernel_spmd(
        nc, _in_maps(x), list(range(N_CORES)), trace=trace, tmpdir=tmpdir
    )
    out = np.empty((B, 1, H, W), np.float32)
    for k in range(N_CORES):
        b, half = divmod(k, 2)
        out[b, 0, half * HALF : (half + 1) * HALF, :] = res.results[k]["out"]
    return out, res


def kernel(x: np.ndarray) -> np.ndarray:
    x = np.ascontiguousarray(x, dtype=np.float32)
    assert x.shape == (B, C, H, W)
    return _run(x)[0]


# revision 8
# speedup vs baseline: 1.1020x; 1.1020x over previous
"""BoxFilter kernel for Trainium2 (8 NeuronCores).

Computes out[b,0,i,j] = sum_{c} sum_{|di|<=15} sum_{|dj|<=15} x[b,c,i+di,j+dj]
(edge-clamped 31x31 box filter over the channel-summed image), matching the
reference cumsum + shifted-diff formulation exactly (separable box sums).

Sharding: data-parallel over (batch, H-half) -> 8 shards, no cross-core
communication. Each core receives a host-padded [3, 1056, 2048] slab
(16 halo rows on each side, zero-filled past the global image edges).

Per-core pipeline (all f32):
  1. channel-sum on DVE (2 adds per 128-row tile)
  2. vertical 31-tap box sum via two banded 0/1-matrix fp32 matmuls per
     PSUM bank (bands are compile-time constant inputs)
  3. ACT copies PSUM -> zero-padded SBUF tile
  4. horizontal 31-tap box sum in a single tensor_tensor_scan:
     state_j = state_{j-1} + xp[j] - xp[j-31]
  5. DMA result rows to DRAM
"""

import numpy as np

R = 15
TAP = 2 * R + 1          # 31
B, C, H, W = 4, 3, 2048, 2048
HALF = H // 2            # 1024 output rows per core
S_ROWS = HALF + 32       # 1056 input rows per core (16-row halo each side)
N_CORES = 8
PAD_L = TAP              # left zero pad for the scan (31)
PAD_R = R                # right zero pad (15)
XP_W = PAD_L + W + PAD_R # 2094
SCAN_N = W + R           # 2063 scan steps; out col j = scan[j + R]
P = 128                  # SBUF partitions
N_OUT_TILES = HALF // P  # 8
TAIL_ROWS = S_ROWS - N_OUT_TILES * P  # 32 valid rows in the 9th s-tile
MM_N = 512               # fp32 moving-operand max / one PSUM bank

_CACHE = {}


def _band_matrices():
    # out row i of a 128-row tile needs halo'd input rows r = i+1 .. i+31
    # (r is the row index within the [s_lo; s_hi] 256-row window).
    k = np.arange(P)[:, None]
    i = np.arange(P)[None, :]
    band_a = ((k >= i + 1) & (k <= i + TAP)).astype(np.float32)          # rows in s_lo
    band_b = ((k + P >= i + 1) & (k + P <= i + TAP)).astype(np.float32)  # rows in s_hi
    return band_a, band_b


def _build_kernel(tc, nc, out, xs, band_a_d, band_b_d, mybir, bass):
    from contextlib import ExitStack

    f32 = mybir.dt.float32
    f32r = mybir.dt.float32r
    add = mybir.AluOpType.add
    sub = mybir.AluOpType.subtract

    with ExitStack() as ctx:
        const_pool = ctx.enter_context(tc.tile_pool(name="const", bufs=1))
        xc_pool = ctx.enter_context(tc.tile_pool(name="xc", bufs=3))
        s_pool = ctx.enter_context(tc.tile_pool(name="s", bufs=4))
        xp_pool = ctx.enter_context(tc.tile_pool(name="xp", bufs=3))
        box_pool = ctx.enter_context(tc.tile_pool(name="box", bufs=3))
        psum_pool = ctx.enter_context(
            tc.tile_pool(name="psum", bufs=8, space=bass.MemorySpace.PSUM)
        )

        band_a = const_pool.tile([P, P], f32r)
        band_b = const_pool.tile([P, P], f32r)
        nc.sync.dma_start(band_a[:], band_a_d)
        nc.sync.dma_start(band_b[:], band_b_d)

        def make_s(u):
            rows = P if u < N_OUT_TILES else TAIL_ROWS
            # one batched DMA for all 3 channels: [rows, 3, W], partition-major
            xc = xc_pool.tile([P, C, W], f32)
            nc.sync.dma_start(
                xc[:rows],
                xs[:, P * u : P * u + rows, :].rearrange("c p n -> p c n"),
            )
            s = s_pool.tile([P, W], f32r)
            if rows < P:
                # rows >= TAIL_ROWS are multiplied by zero band weights but
                # must be finite, and rows 31.. are simply past the image.
                nc.gpsimd.memset(s[:].bitcast(f32), 0.0)
            # alternate the 2-input adds between DVE and GpSimd so neither
            # engine becomes the pipeline gate (GpSimd TT is ~2.5x slower)
            eng = nc.vector if u % 2 == 0 else nc.gpsimd
            eng.tensor_add(s[:rows, :], xc[:rows, 0, :], xc[:rows, 1, :])
            eng.tensor_add(s[:rows, :], s[:rows, :], xc[:rows, 2, :])
            return s

        s_tiles = {0: make_s(0)}
        for t in range(N_OUT_TILES):
            s_tiles[t + 1] = make_s(t + 1)
            s_lo, s_hi = s_tiles.pop(t), s_tiles[t + 1]

            xp = xp_pool.tile([P, XP_W], f32)
            nc.gpsimd.memset(xp[:, 0:PAD_L], 0.0)
            nc.gpsimd.memset(xp[:, PAD_L + W : XP_W], 0.0)

            # all band_a matmuls, then all band_b: minimizes PE weight reloads
            psums = []
            for nb in range(W // MM_N):
                ps = psum_pool.tile([P, MM_N], f32)
                lo_c = s_lo[:, MM_N * nb : MM_N * (nb + 1)]
                nc.tensor.matmul(
                    ps[:], band_a[:], lo_c, start=True, stop=False
                )
                psums.append(ps)
            for nb in range(W // MM_N):
                hi_c = s_hi[:, MM_N * nb : MM_N * (nb + 1)]
                nc.tensor.matmul(
                    psums[nb][:], band_b[:], hi_c,
                    start=False, stop=True,
                )
                nc.scalar.copy(
                    xp[:, PAD_L + MM_N * nb : PAD_L + MM_N * (nb + 1)],
                    psums[nb][:],
                )

            box = box_pool.tile([P, SCAN_N + 1], f32)
            nc.vector.tensor_tensor_scan(
                box[:, 0:SCAN_N],
                xp[:, PAD_L : PAD_L + SCAN_N],
                xp[:, 0:SCAN_N],
                0.0,
                add,
                sub,
            )
            nc.sync.dma_start(out[P * t : P * (t + 1), :], box[:, R : R + W])


def _get_nc():
    if "nc" in _CACHE:
        return _CACHE["nc"]
    import concourse.bass as bass
    import concourse.tile as tile
    from concourse import bacc, mybir

    nc = bacc.Bacc(
        "TRN2", target_bir_lowering=False, debug=False, num_devices=N_CORES
    )
    xs = nc.dram_tensor("xs", [C, S_ROWS, W], mybir.dt.float32, kind="ExternalInput")
    ba = nc.dram_tensor("band_a", [P, P], mybir.dt.float32r, kind="ExternalInput")
    bb = nc.dram_tensor("band_b", [P, P], mybir.dt.float32r, kind="ExternalInput")
    out = nc.dram_tensor("out", [HALF, W], mybir.dt.float32, kind="ExternalOutput")

    with tile.TileContext(nc) as tc:
        _build_kernel(tc, nc, out.ap(), xs.ap(), ba.ap(), bb.ap(), mybir, bass)
    nc.compile()
    _CACHE["nc"] = nc
    return nc


def _in_maps(x):
    band_a, band_b = _band_matrices()
    maps = []
    for k in range(N_CORES):
        b, half = divmod(k, 2)
        h0 = half * HALF
        lo = h0 - 16  # global row of xs row 0
        g0, g1 = max(lo, 0), min(h0 + HALF + 16, H)
        xs = np.zeros((C, S_ROWS, W), np.float32)
        xs[:, g0 - lo : g1 - lo, :] = x[b, :, g0:g1, :]
        maps.append({"xs": xs, "band_a": band_a, "band_b": band_b})
    return maps


def _run(x, trace=False, tmpdir=None):
    from concourse.bass_utils import run_bass_kernel_spmd

    nc = _get_nc()
    res = run_bass_kernel_spmd(
        nc, _in_maps(x), list(range(N_CORES)), trace=trace, tmpdir=tmpdir
    )
    out = np.empty((B, 1, H, W), np.float32)
    for k in range(N_CORES):
        b, half = divmod(k, 2)
        out[b, 0, half * HALF : (half + 1) * HALF, :] = res.results[k]["out"]
    return out, res


def kernel(x: np.ndarray) -> np.ndarray:
    x = np.ascontiguousarray(x, dtype=np.float32)
    assert x.shape == (B, C, H, W)
    return _run(x)[0]


# revision 10
# speedup vs baseline: 1.3055x; 1.1847x over previous
"""BoxFilter kernel for Trainium2 (8 NeuronCores).

Computes out[b,0,i,j] = sum_{c} sum_{|di|<=15} sum_{|dj|<=15} x[b,c,i+di,j+dj]
(edge-clamped 31x31 box filter over the channel-summed image), matching the
reference cumsum + shifted-diff formulation exactly (separable box sums).

Sharding: data-parallel over (batch, H-half) -> 8 shards, no cross-core
communication. Each core receives a host-padded [3, 1056, 2048] slab
(16 halo rows on each side, zero-filled past the global image edges).

Per-core pipeline (all f32):
  1. channel-sum on DVE (2 adds per 128-row tile)
  2. vertical 31-tap box sum via two banded 0/1-matrix fp32 matmuls per
     PSUM bank (bands are compile-time constant inputs)
  3. ACT copies PSUM -> zero-padded SBUF tile
  4. horizontal 31-tap box sum in a single tensor_tensor_scan:
     state_j = state_{j-1} + xp[j] - xp[j-31]
  5. DMA result rows to DRAM
"""

import numpy as np

R = 15
TAP = 2 * R + 1          # 31
B, C, H, W = 4, 3, 2048, 2048
HALF = H // 2            # 1024 output rows per core
S_ROWS = HALF + 32       # 1056 input rows per core (16-row halo each side)
N_CORES = 8
PAD_L = TAP              # left zero pad for the scan (31)
PAD_R = R                # right zero pad (15)
XP_W = PAD_L + W + PAD_R # 2094
SCAN_N = W + R           # 2063 scan steps; out col j = scan[j + R]
P = 128                  # SBUF partitions
N_OUT_TILES = HALF // P  # 8
TAIL_ROWS = S_ROWS - N_OUT_TILES * P  # 32 valid rows in the 9th s-tile
MM_N = 512               # fp32 moving-operand max / one PSUM bank

_CACHE = {}


def _band_matrices():
    # out row i of a 128-row tile needs halo'd input rows r = i+1 .. i+31
    # (r is the row index within the [s_lo; s_hi] 256-row window).
    k = np.arange(P)[:, None]
    i = np.arange(P)[None, :]
    band_a = ((k >= i + 1) & (k <= i + TAP)).astype(np.float32)          # rows in s_lo
    band_b = ((k + P >= i + 1) & (k + P <= i + TAP)).astype(np.float32)  # rows in s_hi
    return band_a, band_b


def _build_kernel(tc, nc, out, xs, band_a_d, band_b_d, mybir, bass):
    from contextlib import ExitStack

    f32 = mybir.dt.float32
    f32r = mybir.dt.float32r
    add = mybir.AluOpType.add
    sub = mybir.AluOpType.subtract

    with ExitStack() as ctx:
        const_pool = ctx.enter_context(tc.tile_pool(name="const", bufs=1))
        xc_pool = ctx.enter_context(tc.tile_pool(name="xc", bufs=3))
        s_pool = ctx.enter_context(tc.tile_pool(name="s", bufs=4))
        xp_pool = ctx.enter_context(tc.tile_pool(name="xp", bufs=3))
        box_pool = ctx.enter_context(tc.tile_pool(name="box", bufs=3))
        psum_pool = ctx.enter_context(
            tc.tile_pool(name="psum", bufs=8, space=bass.MemorySpace.PSUM)
        )

        band_a = const_pool.tile([P, P], f32r)
        band_b = const_pool.tile([P, P], f32r)
        nc.sync.dma_start(band_a[:], band_a_d)
        nc.sync.dma_start(band_b[:], band_b_d)

        def make_s(u):
            rows = P if u < N_OUT_TILES else TAIL_ROWS
            # one batched DMA for all 3 channels: [rows, 3, W], partition-major
            xc = xc_pool.tile([P, C, W], f32)
            # alternate HWDGE rings (sync vs scalar) — a single logical DMA
            # queue tops out well below the per-core HBM bandwidth
            dma_eng = nc.sync if u % 2 == 0 else nc.scalar
            dma_eng.dma_start(
                xc[:rows],
                xs[:, P * u : P * u + rows, :].rearrange("c p n -> p c n"),
            )
            s = s_pool.tile([P, W], f32r)
            if rows < P:
                # rows >= TAIL_ROWS are multiplied by zero band weights but
                # must be finite, and rows 31.. are simply past the image.
                nc.gpsimd.memset(s[:].bitcast(f32), 0.0)
            # alternate the 2-input adds between DVE and GpSimd so neither
            # engine becomes the pipeline gate (GpSimd TT is ~2.5x slower)
            eng = nc.vector if u % 2 == 0 else nc.gpsimd
            eng.tensor_add(s[:rows, :], xc[:rows, 0, :], xc[:rows, 1, :])
            eng.tensor_add(s[:rows, :], s[:rows, :], xc[:rows, 2, :])
            return s

        s_tiles = {0: make_s(0)}
        for t in range(N_OUT_TILES):
            s_tiles[t + 1] = make_s(t + 1)
            s_lo, s_hi = s_tiles.pop(t), s_tiles[t + 1]

            xp = xp_pool.tile([P, XP_W], f32)
            nc.gpsimd.memset(xp[:, 0:PAD_L], 0.0)
            nc.gpsimd.memset(xp[:, PAD_L + W : XP_W], 0.0)

            # all band_a matmuls, then all band_b: minimizes PE weight reloads
            psums = []
            for nb in range(W // MM_N):
                ps = psum_pool.tile([P, MM_N], f32)
                lo_c = s_lo[:, MM_N * nb : MM_N * (nb + 1)]
                nc.tensor.matmul(
                    ps[:], band_a[:], lo_c, start=True, stop=False
                )
                psums.append(ps)
            for nb in range(W // MM_N):
                hi_c = s_hi[:, MM_N * nb : MM_N * (nb + 1)]
                nc.tensor.matmul(
                    psums[nb][:], band_b[:], hi_c,
                    start=False, stop=True,
                )
                nc.scalar.copy(
                    xp[:, PAD_L + MM_N * nb : PAD_L + MM_N * (nb + 1)],
                    psums[nb][:],
                )

            box = box_pool.tile([P, SCAN_N + 1], f32)
            nc.vector.tensor_tensor_scan(
                box[:, 0:SCAN_N],
                xp[:, PAD_L : PAD_L + SCAN_N],
                xp[:, 0:SCAN_N],
                0.0,
                add,
                sub,
            )
            store_eng = nc.scalar if t % 2 == 0 else nc.sync
            store_eng.dma_start(out[P * t : P * (t + 1), :], box[:, R : R + W])


def _get_nc():
    if "nc" in _CACHE:
        return _CACHE["nc"]
    import concourse.bass as bass
    import concourse.tile as tile
    from concourse import bacc, mybir

    nc = bacc.Bacc(
        "TRN2", target_bir_lowering=False, debug=False, num_devices=N_CORES
    )
    xs = nc.dram_tensor("xs", [C, S_ROWS, W], mybir.dt.float32, kind="ExternalInput")
    ba = nc.dram_tensor("band_a", [P, P], mybir.dt.float32r, kind="ExternalInput")
    bb = nc.dram_tensor("band_b", [P, P], mybir.dt.float32r, kind="ExternalInput")
    out = nc.dram_tensor("out", [HALF, W], mybir.dt.float32, kind="ExternalOutput")

    with tile.TileContext(nc) as tc:
        _build_kernel(tc, nc, out.ap(), xs.ap(), ba.ap(), bb.ap(), mybir, bass)
    nc.compile()
    _CACHE["nc"] = nc
    return nc


def _in_maps(x):
    band_a, band_b = _band_matrices()
    maps = []
    for k in range(N_CORES):
        b, half = divmod(k, 2)
        h0 = half * HALF
        lo = h0 - 16  # global row of xs row 0
        g0, g1 = max(lo, 0), min(h0 + HALF + 16, H)
        xs = np.zeros((C, S_ROWS, W), np.float32)
        xs[:, g0 - lo : g1 - lo, :] = x[b, :, g0:g1, :]
        maps.append({"xs": xs, "band_a": band_a, "band_b": band_b})
    return maps


def _run(x, trace=False, tmpdir=None):
    from concourse.bass_utils import run_bass_kernel_spmd

    nc = _get_nc()
    res = run_bass_kernel_spmd(
        nc, _in_maps(x), list(range(N_CORES)), trace=trace, tmpdir=tmpdir
    )
    out = np.empty((B, 1, H, W), np.float32)
    for k in range(N_CORES):
        b, half = divmod(k, 2)
        out[b, 0, half * HALF : (half + 1) * HALF, :] = res.results[k]["out"]
    return out, res


def kernel(x: np.ndarray) -> np.ndarray:
    x = np.ascontiguousarray(x, dtype=np.float32)
    assert x.shape == (B, C, H, W)
    return _run(x)[0]
